# revision 17
# baseline (speedup 1.0000x reference)
"""Trainium2 Bass kernel for nn_AutoAttention (periodic-shift sparse attention).

Exact algebraic collapse of the reference (verified in fp64):
  scores_s = y . roll_s(x) + x.R[:,s] + (s-const terms, softmax-invariant)
      where y = x @ A,  A = (Wq.T@Wk)/sqrt(H),  R[:,s] = roll(Wk.T@bq, -64s)/sqrt(H)
  out = m @ (Wo@Wv).T + (Wo@bv + bo)   where m = sum_s softmax_s * roll_s(x)
This turns 18 H*H projections per token into 2.

Sharding: data-parallel over B: 8 cores x 2 batches (4096 tokens/core).

Fast path (zero q/k biases — the shipped problem) per 128-token tile:
  PE  : yT = A.T x.T (feature-major), score matmuls  Y_s = yT.T @ roll_s(x).T
        (the roll is folded into host-pretransposed xT at two 64-offset
        alignments, so rhs block selection does the roll), m transposes,
        out-projection.
  DVE : diagonal extraction of the 8 score matmuls via fused
        scalar_tensor_tensor(in1=identity, accum_out) reading PSUM,
        part of the q_s = e_s*roll_s(x) scaled copies + pairwise adds.
  ACT : PSUM->SBUF copies (yT, mT), exp+rowsum, out scale by 1/den, q share.
  POOL: q_s scaled copies (tensor_scalar w/ AP scalar) + pairwise add share.
"""

import os
import sys

import numpy as np

for _p in ("/opt/trn_rl_repo",):
    if _p not in sys.path and os.path.isdir(_p):
        sys.path.insert(0, _p)

B, N, H, P = 16, 2048, 512, 64
S = H // P  # 8
NCORES = 8
TOK = (B // NCORES) * N  # tokens per core = 4096
TT = 128  # tokens per tile
NT = TOK // TT  # 32 tiles
NPAIR = NT // 2  # 16 pair-iterations

# engine-assignment knobs (tuned via cost-model grid search)
# q_s producers: indices 0..Q_DVE-1 on DVE, next Q_ACT on ACT, rest on POOL
CFG = {
    "Q_DVE": int(os.environ.get("K_Q_DVE", 2)),
    "Q_ACT": int(os.environ.get("K_Q_ACT", 2)),
    "TREE_POOL": int(os.environ.get("K_TREE_POOL", 2)),  # of the 4 L1 adds
    "NPAIR": int(os.environ.get("K_NPAIR", NPAIR)),
    "U1_POOL": int(os.environ.get("K_U1_POOL", 0)),
    "PS_SC": int(os.environ.get("K_PS_SC", 2)),
    "PS_O": int(os.environ.get("K_PS_O", 1)),
}

_CACHE = {}


def _build(has_qk_bias: bool):
    import concourse.bass as bass
    import concourse.mybir as mybir
    import concourse.tile as tile
    from concourse import bacc
    from concourse.masks import make_identity

    f32 = mybir.dt.float32
    bf16 = mybir.dt.bfloat16
    A = mybir.AluOpType
    AF = mybir.ActivationFunctionType
    PSUM = bass.MemorySpace.PSUM
    ts = bass.ts

    q_dve = CFG["Q_DVE"]
    q_act = CFG["Q_ACT"]
    tree_pool = CFG["TREE_POOL"]
    u1_pool = CFG["U1_POOL"]
    npair = CFG["NPAIR"]
    YW = H + S if has_qk_bias else H

    nc = bacc.Bacc("TRN2", target_bir_lowering=False, debug=False, num_devices=NCORES)

    xb_d = nc.dram_tensor("xb", [TOK, H], bf16, kind="ExternalInput")
    # xt[a][k,p,i,u] = x[token=i*256+u, (128k+p+64a)%512], a in {0,1}
    xt_d = nc.dram_tensor("xt", [2, 4, 128, NPAIR, 2 * TT], bf16, kind="ExternalInput")
    ga_d = nc.dram_tensor("ga", [H, YW], bf16, kind="ExternalInput")
    ut_d = nc.dram_tensor("ut", [H, H], bf16, kind="ExternalInput")
    out_d = nc.dram_tensor("out", [TOK, H], f32, kind="ExternalOutput")

    with tile.TileContext(nc) as tc:
        with (
            tc.tile_pool(name="consts", bufs=1) as cpool,
            tc.tile_pool(name="xbb", bufs=3) as xbpool,
            tc.tile_pool(name="xtb", bufs=3) as xtpool,
            tc.tile_pool(name="ybt", bufs=2) as ybtpool,
            tc.tile_pool(name="q", bufs=2) as qpool,
            tc.tile_pool(name="tr", bufs=2) as trpool,
            tc.tile_pool(name="mt", bufs=2) as mtpool,
            tc.tile_pool(name="ot", bufs=2) as otpool,
            tc.tile_pool(name="scrd", bufs=2) as scrd_pool,
            tc.tile_pool(name="small", bufs=3) as spool,
            tc.tile_pool(name="ps_y", bufs=1, space=PSUM) as ps_y,
            tc.tile_pool(name="ps_sc", bufs=CFG["PS_SC"], space=PSUM) as ps_sc,
            tc.tile_pool(name="ps_mt", bufs=1, space=PSUM) as ps_mt,
            tc.tile_pool(name="ps_o", bufs=CFG["PS_O"], space=PSUM) as ps_o,
        ):
            gab = cpool.tile([128, 4, YW], bf16)
            nc.sync.dma_start(gab[:], ga_d.ap().rearrange("(k p) o -> p k o", p=128))
            utb = cpool.tile([128, 4, H], bf16)
            nc.sync.dma_start(utb[:], ut_d.ap().rearrange("(k p) o -> p k o", p=128))
            identb = cpool.tile([128, 128], bf16)
            make_identity(nc, identb[:])

            for i in range(npair):
                rows = slice(i * 2 * TT, (i + 1) * 2 * TT)
                xbb = xbpool.tile([128, 2, 2 * H], bf16)
                nc.sync.dma_start(
                    xbb[:, :, 0:H],
                    xb_d[rows, :].rearrange("(h p) c -> p h c", p=128),
                )
                nc.scalar.dma_start(xbb[:, :, H : 2 * H], xbb[:, :, 0:H])
                # both 64-offset alignments of xT
                xtb = xtpool.tile([128, 2, 4, 2 * TT], bf16)
                nc.sync.dma_start(
                    xtb[:], xt_d[:, :, :, i, :].rearrange("a k p u -> p a k u")
                )

                sc = spool.tile([128, 2, S], f32)
                e = spool.tile([128, 2, S], f32)
                den = spool.tile([128, 2], f32)
                rec = spool.tile([128, 2], f32)
                mtp = ps_mt.tile([128, 2, H], bf16)
                ops = ps_o.tile([128, 2, H], f32)
                mt = mtpool.tile([128, 2, H], bf16)
                ot = otpool.tile([128, 2, H], f32)

                for h in range(2):
                    # --- yT projection, feature-major: yT[o,t] (PE) ---
                    ytps = ps_y.tile([128, H], f32, name="ytps", tag="ytps")
                    for ob in range(4):
                        for j in range(4):
                            nc.tensor.matmul(
                                ytps[:, ts(ob, 128)],
                                gab[:, j, ts(ob, 128)],
                                xtb[:, 0, j, ts(h, TT)],
                                start=(j == 0),
                                stop=(j == 3),
                            )
                    ybt = ybtpool.tile([128, 4, 128], bf16, name="ybt", tag="ybt")
                    nc.scalar.copy(ybt[:], ytps[:])
                    if has_qk_bias:
                        # sbias[t,s] = x[t] . R[:,s] : project, then PE-transpose
                        sbp = ps_y.tile([8, 128], f32, name="sbp", tag="sbp")
                        for j in range(4):
                            nc.tensor.matmul(
                                sbp[:],
                                gab[:, j, H:YW],
                                xtb[:, 0, j, ts(h, TT)],
                                start=(j == 0),
                                stop=(j == 3),
                            )
                        sbb = ybtpool.tile([8, 128], bf16, name="sbb", tag="sbb")
                        nc.scalar.copy(sbb[:], sbp[:])
                        sbt = ps_y.tile([128, 8], bf16, name="sbt", tag="sbt")
                        nc.tensor.transpose(sbt[:], sbb[:], identb[0:8, 0:8])

                    # --- score matmuls: S_s[t,t'] = sum_g yT[g,t]*xT[(g-64s),t']
                    # roll folded into rhs block choice (alignment a = s%2) ---
                    scp = ps_sc.tile([128, S * 128], f32, name="scp", tag="scp")
                    for s in range(S):
                        a = s % 2
                        for j in range(4):
                            jb = (j - (s + 1) // 2) % 4 if a else (j - s // 2) % 4
                            nc.tensor.matmul(
                                scp[:, ts(s, 128)],
                                ybt[:, j, :],
                                xtb[:, a, jb, ts(h, TT)],
                                start=(j == 0),
                                stop=(j == 3),
                                skip_group_check=True,
                            )
                    # --- diagonal extraction (DVE, fused STT with accum) ---
                    scrd = scrd_pool.tile([128, 128], bf16, name="scrd", tag="scrd")
                    for s in range(S):
                        nc.vector.scalar_tensor_tensor(
                            out=scrd[:],
                            in0=scp[:, ts(s, 128)],
                            scalar=1.0,
                            in1=identb[:],
                            op0=A.mult,
                            op1=A.mult,
                            accum_out=sc[:, h, s : s + 1],
                        )

                    if has_qk_bias:
                        nc.vector.tensor_tensor(
                            out=sc[:, h, :], in0=sc[:, h, :], in1=sbt[:],
                            op=A.add,
                        )
                    # --- softmax pieces (ACT) ---
                    nc.scalar.activation(
                        out=e[:, h, :],
                        in_=sc[:, h, :],
                        func=AF.Exp,
                        accum_out=den[:, h : h + 1],
                    )

                    # --- q_s = e_s * roll_s(x), bf16 scaled copies, 3 engines ---
                    qs = []
                    for s in range(S):
                        q = qpool.tile(
                            [128, H], bf16, name=f"q{s}", tag=f"q{s}"
                        )
                        src = xbb[:, h, H - P * s : 2 * H - P * s]
                        ev = e[:, h, s : s + 1]
                        if s < q_dve:
                            nc.vector.tensor_scalar(
                                out=q[:], in0=src, scalar1=ev, scalar2=None,
                                op0=A.mult,
                            )
                        elif s < q_dve + q_act:
                            nc.scalar.activation(
                                out=q[:], in_=src, func=AF.Copy, scale=ev
                            )
                        else:
                            nc.gpsimd.tensor_scalar(
                                out=q[:], in0=src, scalar1=ev, scalar2=None,
                                op0=A.mult,
                            )
                        qs.append(q)

                    # --- pairwise-add tree (DVE/POOL), bf16 ---
                    rr = []
                    for k in range(4):
                        r = trpool.tile([128, H], bf16, name=f"r{k}", tag=f"r{k}")
                        eng = nc.gpsimd if k >= 4 - tree_pool else nc.vector
                        eng.tensor_tensor(
                            out=r[:], in0=qs[2 * k][:], in1=qs[2 * k + 1][:],
                            op=A.add,
                        )
                        rr.append(r)
                    u0 = trpool.tile([128, H], bf16, name="u0", tag="u0")
                    nc.vector.tensor_tensor(
                        out=u0[:], in0=rr[0][:], in1=rr[1][:], op=A.add
                    )
                    u1 = trpool.tile([128, H], bf16, name="u1", tag="u1")
                    (nc.gpsimd if u1_pool else nc.vector).tensor_tensor(
                        out=u1[:], in0=rr[2][:], in1=rr[3][:], op=A.add
                    )
                    mb = trpool.tile([128, H], bf16, name="mb", tag="mb")
                    nc.vector.tensor_tensor(
                        out=mb[:], in0=u0[:], in1=u1[:], op=A.add
                    )

                    # --- transpose m (PE, bf16, no accumulation) ---
                    for j in range(4):
                        nc.tensor.transpose(
                            mtp[:, h, ts(j, 128)], mb[:, ts(j, 128)], identb[:]
                        )

                nc.vector.reciprocal(rec[:], den[:])
                nc.scalar.copy(mt[:], mtp[:])

                for h in range(2):
                    for j in range(4):
                        nc.tensor.matmul(
                            ops[:, h, :],
                            mt[:, h, ts(j, 128)],
                            utb[:, j, :],
                            start=(j == 0),
                            stop=(j == 3),
                        )
                    nc.scalar.activation(
                        out=ot[:, h, :],
                        in_=ops[:, h, :],
                        func=AF.Copy,
                        scale=rec[:, h : h + 1],
                    )
                nc.sync.dma_start(
                    out_d[rows, :].rearrange("(h p) c -> p h c", p=128), ot[:]
                )

    nc.compile()
    return nc


def _get_nc(has_qk_bias: bool = False):
    key = ("nc", has_qk_bias) + tuple(sorted(CFG.items()))
    if key not in _CACHE:
        _CACHE[key] = _build(has_qk_bias)
    return _CACHE[key]


def _prep(hidden_states, Wq, bq, Wk, bk, Wv, bv, Wo, bo):
    """Host-side: shard x, fold weights, pre-transpose/cast."""
    import ml_dtypes

    bf16 = ml_dtypes.bfloat16
    x = np.ascontiguousarray(np.asarray(hidden_states, dtype=np.float32)).reshape(
        NCORES, TOK, H
    )
    Wq, Wk, Wv, Wo = (np.asarray(w, dtype=np.float64) for w in (Wq, Wk, Wv, Wo))
    bq, bk, bv, bo = (np.asarray(b, dtype=np.float64) for b in (bq, bk, bv, bo))

    has_qk_bias = bool(np.any(bq != 0.0))
    inv = 1.0 / np.sqrt(H)
    Amat = (Wq.T @ Wk) * inv  # [H,H]
    if has_qk_bias:
        wv_ = Wk.T @ bq
        R = np.stack([np.roll(wv_, -P * s) for s in range(S)], axis=1) * inv
        ga = np.concatenate([Amat, R], axis=1)
    else:
        ga = Amat
    ga = np.ascontiguousarray(ga.astype(bf16))
    ut = np.ascontiguousarray((Wo @ Wv).T.astype(bf16))  # [H,H]
    c_out = (Wo @ bv + bo).astype(np.float32)  # [H]

    xbf = x.astype(bf16)  # [NC, TOK, H]
    # xt[c][a][k,p,i,u] = x[c, token=i*256+u, (128k+p+64a)%512]
    xt0 = xbf.reshape(NCORES, NPAIR, 2 * TT, 4, 128).transpose(0, 3, 4, 1, 2)
    xr = np.roll(xbf, -64, axis=2)  # x[..., (g+64)%512]
    xt1 = xr.reshape(NCORES, NPAIR, 2 * TT, 4, 128).transpose(0, 3, 4, 1, 2)
    xt = np.ascontiguousarray(np.stack([xt0, xt1], axis=1))
    in_maps = [
        {"xb": xbf[c], "xt": xt[c], "ga": ga, "ut": ut} for c in range(NCORES)
    ]
    return in_maps, c_out, has_qk_bias


def _run(in_maps, has_qk_bias=False, trace=False):
    from concourse.bass_utils import run_bass_kernel_spmd

    nc = _get_nc(has_qk_bias)
    return run_bass_kernel_spmd(nc, in_maps, core_ids=list(range(NCORES)), trace=trace)


def kernel(hidden_states, Wq, bq, Wk, bk, Wv, bv, Wo, bo):
    in_maps, c_out, has_qk_bias = _prep(
        hidden_states, Wq, bq, Wk, bk, Wv, bv, Wo, bo
    )
    res = _run(in_maps, has_qk_bias)
    out = np.stack([r["out"] for r in res.results], axis=0).reshape(B, N, H)
    return (out + c_out[None, None, :]).astype(np.float32)


# revision 18
# speedup vs baseline: 1.1103x; 1.1103x over previous
"""Trainium2 Bass kernel for nn_AutoAttention (periodic-shift sparse attention).

Exact algebraic collapse of the reference (verified in fp64):
  scores_s = y . roll_s(x) + x.R[:,s] + (s-const terms, softmax-invariant)
      where y = x @ A,  A = (Wq.T@Wk)/sqrt(H),  R[:,s] = roll(Wk.T@bq, -64s)/sqrt(H)
  out = m @ (Wo@Wv).T + (Wo@bv + bo)   where m = sum_s softmax_s * roll_s(x)
This turns 18 H*H projections per token into 2.

Sharding: data-parallel over B: 8 cores x 2 batches (4096 tokens/core).

Fast path (zero q/k biases — the shipped problem) per 128-token tile:
  PE  : yT = A.T x.T (feature-major), score matmuls  Y_s = yT.T @ roll_s(x).T
        (the roll is folded into host-pretransposed xT at two 64-offset
        alignments, so rhs block selection does the roll), m transposes,
        out-projection.
  DVE : diagonal extraction of the 8 score matmuls via fused
        scalar_tensor_tensor(in1=identity, accum_out) reading PSUM,
        part of the q_s = e_s*roll_s(x) scaled copies + pairwise adds.
  ACT : PSUM->SBUF copies (yT, mT), exp+rowsum, out scale by 1/den, q share.
  POOL: q_s scaled copies (tensor_scalar w/ AP scalar) + pairwise add share.
"""

import os
import sys

import numpy as np

for _p in ("/opt/trn_rl_repo",):
    if _p not in sys.path and os.path.isdir(_p):
        sys.path.insert(0, _p)

B, N, H, P = 16, 2048, 512, 64
S = H // P  # 8
NCORES = 8
TOK = (B // NCORES) * N  # tokens per core = 4096
TT = 128  # tokens per tile
NT = TOK // TT  # 32 tiles
NPAIR = NT // 2  # 16 pair-iterations

# engine-assignment knobs (tuned via cost-model grid search)
# q_s producers: indices 0..Q_DVE-1 on DVE, next Q_ACT on ACT, rest on POOL
CFG = {
    "Q_DVE": int(os.environ.get("K_Q_DVE", 2)),
    "Q_ACT": int(os.environ.get("K_Q_ACT", 0)),
    "TREE_POOL": int(os.environ.get("K_TREE_POOL", 2)),  # of the 4 L1 adds
    "NPAIR": int(os.environ.get("K_NPAIR", NPAIR)),
    "U1_POOL": int(os.environ.get("K_U1_POOL", 0)),
    "PS_SC": int(os.environ.get("K_PS_SC", 2)),
    "PS_O": int(os.environ.get("K_PS_O", 1)),
}

_CACHE = {}


def _build(has_qk_bias: bool):
    import concourse.bass as bass
    import concourse.mybir as mybir
    import concourse.tile as tile
    from concourse import bacc
    from concourse.masks import make_identity

    f32 = mybir.dt.float32
    bf16 = mybir.dt.bfloat16
    A = mybir.AluOpType
    AF = mybir.ActivationFunctionType
    PSUM = bass.MemorySpace.PSUM
    ts = bass.ts

    q_dve = CFG["Q_DVE"]
    q_act = CFG["Q_ACT"]
    tree_pool = CFG["TREE_POOL"]
    u1_pool = CFG["U1_POOL"]
    npair = CFG["NPAIR"]
    YW = H + S if has_qk_bias else H

    nc = bacc.Bacc("TRN2", target_bir_lowering=False, debug=False, num_devices=NCORES)

    xb_d = nc.dram_tensor("xb", [TOK, H], bf16, kind="ExternalInput")
    # xt[a][k,p,i,u] = x[token=i*256+u, (128k+p+64a)%512], a in {0,1}
    xt_d = nc.dram_tensor("xt", [2, 4, 128, NPAIR, 2 * TT], bf16, kind="ExternalInput")
    ga_d = nc.dram_tensor("ga", [H, YW], bf16, kind="ExternalInput")
    ut_d = nc.dram_tensor("ut", [H, H], bf16, kind="ExternalInput")
    out_d = nc.dram_tensor("out", [TOK, H], f32, kind="ExternalOutput")

    with tile.TileContext(nc) as tc:
        with (
            tc.tile_pool(name="consts", bufs=1) as cpool,
            tc.tile_pool(name="xbb", bufs=3) as xbpool,
            tc.tile_pool(name="xtb", bufs=3) as xtpool,
            tc.tile_pool(name="ybt", bufs=2) as ybtpool,
            tc.tile_pool(name="q", bufs=2) as qpool,
            tc.tile_pool(name="tr", bufs=2) as trpool,
            tc.tile_pool(name="mt", bufs=2) as mtpool,
            tc.tile_pool(name="ot", bufs=2) as otpool,
            tc.tile_pool(name="scrd", bufs=2) as scrd_pool,
            tc.tile_pool(name="small", bufs=3) as spool,
            tc.tile_pool(name="ps_y", bufs=1, space=PSUM) as ps_y,
            tc.tile_pool(name="ps_sc", bufs=CFG["PS_SC"], space=PSUM) as ps_sc,
            tc.tile_pool(name="ps_mt", bufs=1, space=PSUM) as ps_mt,
            tc.tile_pool(name="ps_o", bufs=CFG["PS_O"], space=PSUM) as ps_o,
        ):
            gab = cpool.tile([128, 4, YW], bf16)
            nc.sync.dma_start(gab[:], ga_d.ap().rearrange("(k p) o -> p k o", p=128))
            utb = cpool.tile([128, 4, H], bf16)
            nc.sync.dma_start(utb[:], ut_d.ap().rearrange("(k p) o -> p k o", p=128))
            identb = cpool.tile([128, 128], bf16)
            make_identity(nc, identb[:])

            for i in range(npair):
                rows = slice(i * 2 * TT, (i + 1) * 2 * TT)
                xbb = xbpool.tile([128, 2, 2 * H], bf16)
                nc.sync.dma_start(
                    xbb[:, :, 0:H],
                    xb_d[rows, :].rearrange("(h p) c -> p h c", p=128),
                )
                nc.scalar.dma_start(xbb[:, :, H : 2 * H], xbb[:, :, 0:H])
                # both 64-offset alignments of xT
                xtb = xtpool.tile([128, 2, 4, 2 * TT], bf16)
                nc.sync.dma_start(
                    xtb[:], xt_d[:, :, :, i, :].rearrange("a k p u -> p a k u")
                )

                sc = spool.tile([128, 2, S], f32)
                e = spool.tile([128, 2, S], f32)
                den = spool.tile([128, 2], f32)
                rec = spool.tile([128, 2], f32)
                mtp = ps_mt.tile([128, 2, H], bf16)
                ops = ps_o.tile([128, 2, H], f32)
                mt = mtpool.tile([128, 2, H], bf16)
                ot = otpool.tile([128, 2, H], f32)

                for h in range(2):
                    # --- yT projection, feature-major: yT[o,t] (PE) ---
                    ytps = ps_y.tile([128, H], f32, name="ytps", tag="ytps")
                    for ob in range(4):
                        for j in range(4):
                            nc.tensor.matmul(
                                ytps[:, ts(ob, 128)],
                                gab[:, j, ts(ob, 128)],
                                xtb[:, 0, j, ts(h, TT)],
                                start=(j == 0),
                                stop=(j == 3),
                            )
                    ybt = ybtpool.tile([128, 4, 128], bf16, name="ybt", tag="ybt")
                    nc.scalar.copy(ybt[:], ytps[:])
                    if has_qk_bias:
                        # sbias[t,s] = x[t] . R[:,s] : project, then PE-transpose
                        sbp = ps_y.tile([8, 128], f32, name="sbp", tag="sbp")
                        for j in range(4):
                            nc.tensor.matmul(
                                sbp[:],
                                gab[:, j, H:YW],
                                xtb[:, 0, j, ts(h, TT)],
                                start=(j == 0),
                                stop=(j == 3),
                            )
                        sbb = ybtpool.tile([8, 128], bf16, name="sbb", tag="sbb")
                        nc.scalar.copy(sbb[:], sbp[:])
                        sbt = ps_y.tile([128, 8], bf16, name="sbt", tag="sbt")
                        nc.tensor.transpose(sbt[:], sbb[:], identb[0:8, 0:8])

                    # --- score matmuls: S_s[t,t'] = sum_g yT[g,t]*xT[(g-64s),t']
                    # roll folded into rhs block choice (alignment a = s%2) ---
                    scp = ps_sc.tile([128, S * 128], f32, name="scp", tag="scp")
                    for s in range(S):
                        a = s % 2
                        for j in range(4):
                            jb = (j - (s + 1) // 2) % 4 if a else (j - s // 2) % 4
                            nc.tensor.matmul(
                                scp[:, ts(s, 128)],
                                ybt[:, j, :],
                                xtb[:, a, jb, ts(h, TT)],
                                start=(j == 0),
                                stop=(j == 3),
                                skip_group_check=True,
                            )
                    # --- diagonal extraction (DVE, fused STT with accum) ---
                    scrd = scrd_pool.tile([128, 128], bf16, name="scrd", tag="scrd")
                    for s in range(S):
                        nc.vector.scalar_tensor_tensor(
                            out=scrd[:],
                            in0=scp[:, ts(s, 128)],
                            scalar=1.0,
                            in1=identb[:],
                            op0=A.mult,
                            op1=A.mult,
                            accum_out=sc[:, h, s : s + 1],
                        )

                    if has_qk_bias:
                        nc.vector.tensor_tensor(
                            out=sc[:, h, :], in0=sc[:, h, :], in1=sbt[:],
                            op=A.add,
                        )
                    # --- softmax pieces (ACT) ---
                    nc.scalar.activation(
                        out=e[:, h, :],
                        in_=sc[:, h, :],
                        func=AF.Exp,
                        accum_out=den[:, h : h + 1],
                    )

                    # --- q_s = e_s * roll_s(x), bf16 scaled copies, 3 engines ---
                    qs = []
                    for s in range(S):
                        q = qpool.tile(
                            [128, H], bf16, name=f"q{s}", tag=f"q{s}"
                        )
                        src = xbb[:, h, H - P * s : 2 * H - P * s]
                        ev = e[:, h, s : s + 1]
                        if s < q_dve:
                            nc.vector.tensor_scalar(
                                out=q[:], in0=src, scalar1=ev, scalar2=None,
                                op0=A.mult,
                            )
                        elif s < q_dve + q_act:
                            nc.scalar.activation(
                                out=q[:], in_=src, func=AF.Copy, scale=ev
                            )
                        else:
                            nc.gpsimd.tensor_scalar(
                                out=q[:], in0=src, scalar1=ev, scalar2=None,
                                op0=A.mult,
                            )
                        qs.append(q)

                    # --- pairwise-add tree (DVE/POOL), bf16 ---
                    rr = []
                    for k in range(4):
                        r = trpool.tile([128, H], bf16, name=f"r{k}", tag=f"r{k}")
                        eng = nc.gpsimd if k >= 4 - tree_pool else nc.vector
                        eng.tensor_tensor(
                            out=r[:], in0=qs[2 * k][:], in1=qs[2 * k + 1][:],
                            op=A.add,
                        )
                        rr.append(r)
                    u0 = trpool.tile([128, H], bf16, name="u0", tag="u0")
                    nc.vector.tensor_tensor(
                        out=u0[:], in0=rr[0][:], in1=rr[1][:], op=A.add
                    )
                    u1 = trpool.tile([128, H], bf16, name="u1", tag="u1")
                    (nc.gpsimd if u1_pool else nc.vector).tensor_tensor(
                        out=u1[:], in0=rr[2][:], in1=rr[3][:], op=A.add
                    )
                    mb = trpool.tile([128, H], bf16, name="mb", tag="mb")
                    nc.vector.tensor_tensor(
                        out=mb[:], in0=u0[:], in1=u1[:], op=A.add
                    )

                    # --- transpose m (PE, bf16, no accumulation) ---
                    for j in range(4):
                        nc.tensor.transpose(
                            mtp[:, h, ts(j, 128)], mb[:, ts(j, 128)], identb[:]
                        )

                nc.vector.reciprocal(rec[:], den[:])
                nc.scalar.copy(mt[:], mtp[:])

                for h in range(2):
                    for j in range(4):
                        nc.tensor.matmul(
                            ops[:, h, :],
                            mt[:, h, ts(j, 128)],
                            utb[:, j, :],
                            start=(j == 0),
                            stop=(j == 3),
                        )
                    nc.scalar.activation(
                        out=ot[:, h, :],
                        in_=ops[:, h, :],
                        func=AF.Copy,
                        scale=rec[:, h : h + 1],
                    )
                nc.sync.dma_start(
                    out_d[rows, :].rearrange("(h p) c -> p h c", p=128), ot[:]
                )

    nc.compile()
    return nc


def _get_nc(has_qk_bias: bool = False):
    key = ("nc", has_qk_bias) + tuple(sorted(CFG.items()))
    if key not in _CACHE:
        _CACHE[key] = _build(has_qk_bias)
    return _CACHE[key]


def _prep(hidden_states, Wq, bq, Wk, bk, Wv, bv, Wo, bo):
    """Host-side: shard x, fold weights, pre-transpose/cast."""
    import ml_dtypes

    bf16 = ml_dtypes.bfloat16
    x = np.ascontiguousarray(np.asarray(hidden_states, dtype=np.float32)).reshape(
        NCORES, TOK, H
    )
    Wq, Wk, Wv, Wo = (np.asarray(w, dtype=np.float64) for w in (Wq, Wk, Wv, Wo))
    bq, bk, bv, bo = (np.asarray(b, dtype=np.float64) for b in (bq, bk, bv, bo))

    has_qk_bias = bool(np.any(bq != 0.0))
    inv = 1.0 / np.sqrt(H)
    Amat = (Wq.T @ Wk) * inv  # [H,H]
    if has_qk_bias:
        wv_ = Wk.T @ bq
        R = np.stack([np.roll(wv_, -P * s) for s in range(S)], axis=1) * inv
        ga = np.concatenate([Amat, R], axis=1)
    else:
        ga = Amat
    ga = np.ascontiguousarray(ga.astype(bf16))
    ut = np.ascontiguousarray((Wo @ Wv).T.astype(bf16))  # [H,H]
    c_out = (Wo @ bv + bo).astype(np.float32)  # [H]

    xbf = x.astype(bf16)  # [NC, TOK, H]
    # xt[c][a][k,p,i,u] = x[c, token=i*256+u, (128k+p+64a)%512]
    xt0 = xbf.reshape(NCORES, NPAIR, 2 * TT, 4, 128).transpose(0, 3, 4, 1, 2)
    xr = np.roll(xbf, -64, axis=2)  # x[..., (g+64)%512]
    xt1 = xr.reshape(NCORES, NPAIR, 2 * TT, 4, 128).transpose(0, 3, 4, 1, 2)
    xt = np.ascontiguousarray(np.stack([xt0, xt1], axis=1))
    in_maps = [
        {"xb": xbf[c], "xt": xt[c], "ga": ga, "ut": ut} for c in range(NCORES)
    ]
    return in_maps, c_out, has_qk_bias


def _run(in_maps, has_qk_bias=False, trace=False):
    from concourse.bass_utils import run_bass_kernel_spmd

    nc = _get_nc(has_qk_bias)
    return run_bass_kernel_spmd(nc, in_maps, core_ids=list(range(NCORES)), trace=trace)


def kernel(hidden_states, Wq, bq, Wk, bk, Wv, bv, Wo, bo):
    in_maps, c_out, has_qk_bias = _prep(
        hidden_states, Wq, bq, Wk, bk, Wv, bv, Wo, bo
    )
    res = _run(in_maps, has_qk_bias)
    out = np.stack([r["out"] for r in res.results], axis=0).reshape(B, N, H)
    return (out + c_out[None, None, :]).astype(np.float32)
